# revision 2
# baseline (speedup 1.0000x reference)
"""ProbAttention (sparse attention) Trainium2 kernel.

Reference computation per (b, h):
    QK_s = Q @ K_even^T                       [81, 41]   (even-indexed keys)
    M    = QK_s.max(-1) - QK_s.sum(-1)/81     [81]
    top10 = top_k(M, 10) indices              (descending M)
    scores = (Q[top10] @ K^T + rpb[:10]) / 8  [10, 81]
    attn = softmax(scores, -1)                [10, 81]
    ctx  = cumsum(V, seq); ctx[top10] = attn @ V
Outputs: (ctx swapped to [B, L, H, D], attn [B, H, 10, 81])

Device strategy (pure data parallel over B, 128 window-batches per core):
  - Host pre-transposes Q, K into [d, l] ("T") layouts so the d-contraction
    matmuls need no on-device transposes.
  - Phase 1: per (b,h) matmul Qt^T @ [K_even | -Ksum/81] -> [81, 42] PSUM;
    grouped free-dim reduce gives M for 8 heads at once; per 16-b chunk the
    [81, 128] M matrix is PE-transposed and top-10 is found with the DVE
    max8/max_index/match_replace ops (two passes).
  - Top-10 query rows are fetched with an indirect (gather) DMA and
    PE-transposed back into [d, u] operand layout.
  - scores are computed transposed ([81, u] per pair, batched 12 pairs per
    PSUM bank along the free dim), bias-added, PE-transposed to [u*12, 81]
    and softmaxed in batch (ACT exp with fused scale/bias/accum-sum).
  - upd = attn @ V computed transposed ([64, u*12] batched), normalized by
    the softmax denominator during PSUM evacuation.
  - ctx cumsum is one lower-triangular matmul per b: L^T(tri) @ V -> [81, 512].
  - The 10-row scatter of upd into ctx is done on the host during unsharding
    (device returns dense ctx, upd rows, and the top-10 indices).
"""

import sys

sys.path.insert(0, "/opt/trn_rl_repo")

from contextlib import ExitStack

import numpy as np

from concourse import bacc, bass, mybir, tile
from concourse.bass import IndirectOffsetOnAxis
from concourse.bass_utils import run_bass_kernel_spmd
from concourse.masks import make_identity

B, L, H, D = 1024, 81, 8, 64
NCORES = 8
WS = 9
U = 10          # n_top queries
S = 41          # sampled (even) keys
SX = S + 1      # + folded -sum/81 column
HD = H * D      # 512
GMAX = 12       # pairs per softmax group (12*10=120 <= 128 partitions)

F32 = mybir.dt.float32
I32 = mybir.dt.int32
U32 = mybir.dt.uint32
AX = mybir.AxisListType
ALU = mybir.AluOpType
ACTF = mybir.ActivationFunctionType


def _rel_pos_index(ws):
    coords = np.stack(np.meshgrid(np.arange(ws), np.arange(ws), indexing="ij"))
    cf = coords.reshape(2, -1)
    rel = (cf[:, :, None] - cf[:, None, :]).transpose(1, 2, 0)
    rel[..., 0] += ws - 1
    rel[..., 1] += ws - 1
    rel[..., 0] *= 2 * ws - 1
    return rel.sum(-1)


def build_program(bs):
    """Build the SPMD Bass program for a per-core shard of `bs` batches."""
    ch_b = min(16, bs)          # batches per chunk
    assert bs % ch_b == 0
    nch = bs // ch_b
    P = ch_b * H                # (b, h) pairs per chunk (<= 128)

    nc = bacc.Bacc("TRN2", target_bir_lowering=False, debug=False,
                   num_devices=NCORES)

    qt_d = nc.dram_tensor("qt_in", [bs, D, H * L], F32, kind="ExternalInput").ap()
    ktx_d = nc.dram_tensor("ktx_in", [bs, D, H * SX], F32, kind="ExternalInput").ap()
    kt_d = nc.dram_tensor("kt_in", [bs, D, H * L], F32, kind="ExternalInput").ap()
    v_d = nc.dram_tensor("v_in", [bs, L, HD], F32, kind="ExternalInput").ap()
    qg_d = nc.dram_tensor("qg_in", [bs * H * L, D], F32, kind="ExternalInput").ap()
    rpbt_d = nc.dram_tensor("rpbt_in", [L, U * GMAX], F32, kind="ExternalInput").ap()
    ltri_d = nc.dram_tensor("ltri_in", [L, L], F32, kind="ExternalInput").ap()

    ctx_d = nc.dram_tensor("ctx_out", [bs * L, HD], F32, kind="ExternalOutput").ap()
    attn_d = nc.dram_tensor("attn_out", [bs * H * U, L], F32, kind="ExternalOutput").ap()
    upd_d = nc.dram_tensor("upd_out", [bs * H * U, D], F32, kind="ExternalOutput").ap()
    idx_d = nc.dram_tensor("idx_out", [bs * H, U], F32, kind="ExternalOutput").ap()

    with tile.TileContext(nc) as tc, ExitStack() as ctx:
        pool = lambda name, bufs, space="SBUF": ctx.enter_context(
            tc.tile_pool(name=name, bufs=bufs, space=space))

        const_p = pool("const", 1)
        qt_p = pool("qt", 3)
        ktx_p = pool("ktx", 3)
        kt_p = pool("kt", ch_b + 2)
        v_p = pool("v", ch_b + 2)
        mx_p = pool("mx", 3)
        mc_p = pool("mc", 2)
        tk_p = pool("tk", 2)
        gath_p = pool("gath", 2)
        qrt_p = pool("qrt", 2)
        sm_p = pool("sm", 3)
        ctxsb_p = pool("ctxsb", 3)

        qks_pp = pool("qks_pp", 2, "PSUM")
        cum_pp = pool("cum_pp", 2, "PSUM")
        grp_pp = pool("grp_pp", 4, "PSUM")

        ident = const_p.tile([128, 128], F32, tag="ident")
        make_identity(nc, ident[:])
        rpbt = const_p.tile([L, U * GMAX], F32, tag="rpbt")
        nc.sync.dma_start(rpbt[:], rpbt_d[:])
        ltri = const_p.tile([L, L], F32, tag="ltri")
        nc.sync.dma_start(ltri[:], ltri_d[:])

        for c in range(nch):
            mcols = mc_p.tile([L, 128], F32, tag="mcols")
            kts = []
            vbs = []
            for bl in range(ch_b):
                b = c * ch_b + bl
                qt = qt_p.tile([D, H * L], F32, tag="qt")
                nc.sync.dma_start(qt[:], qt_d[b])
                ktx = ktx_p.tile([D, H * SX], F32, tag="ktx")
                nc.sync.dma_start(ktx[:], ktx_d[b])
                kt = kt_p.tile([D, H * L], F32, tag="kt")
                nc.sync.dma_start(kt[:], kt_d[b])
                vb = v_p.tile([L, HD], F32, tag="v")
                nc.sync.dma_start(vb[:], v_d[b])
                kts.append(kt)
                vbs.append(vb)

                # Phase 1: QK over sampled keys (+ -sum/81 in col 41)
                qks = qks_pp.tile([L, H * SX], F32, tag="qks")
                for h in range(H):
                    nc.tensor.matmul(
                        qks[:, h * SX:(h + 1) * SX],
                        lhsT=qt[:, h * L:(h + 1) * L],
                        rhs=ktx[:, h * SX:(h + 1) * SX],
                        start=True, stop=True)
                qksv = qks[:].rearrange("p (h s) -> p h s", s=SX)
                mxt = mx_p.tile([L, H], F32, tag="mxt")
                nc.vector.reduce_max(mxt[:], qksv[:, :, 0:S], axis=AX.X)
                # M = max + (-sum/81): col 41 of each head block
                nc.vector.tensor_add(
                    mcols[:, bl * H:(bl + 1) * H], mxt[:], qksv[:, :, S])

                # Independent: causal cumsum of V via lower-tri matmul
                cum = cum_pp.tile([L, HD], F32, tag="cum")
                nc.tensor.matmul(cum[:], lhsT=ltri[:], rhs=vb[:],
                                 start=True, stop=True)
                ctxsb = ctxsb_p.tile([L, HD], F32, tag="ctxsb")
                nc.scalar.copy(ctxsb[:], cum[:])
                nc.sync.dma_start(ctx_d[b * L:(b + 1) * L], ctxsb[:])

            # ---- top-10 per pair over the chunk ----
            mt_ps = grp_pp.tile([128, L], F32, tag="grp")
            nc.tensor.transpose(mt_ps[:P, :], mcols[:, :P], ident[:L, :L])
            xsb = tk_p.tile([128, L], F32, tag="xsb")
            nc.vector.tensor_copy(xsb[:P], mt_ps[:P])
            mx8 = tk_p.tile([128, 8], F32, tag="mx8")
            nc.vector.max(out=mx8[:P], in_=xsb[:P])
            idx1 = tk_p.tile([128, 8], U32, tag="idx1")
            nc.vector.max_index(idx1[:P], mx8[:P], xsb[:P])
            x2 = tk_p.tile([128, L], F32, tag="x2")
            nc.vector.match_replace(out=x2[:P], in_to_replace=mx8[:P],
                                    in_values=xsb[:P], imm_value=-1e30)
            mx8b = tk_p.tile([128, 8], F32, tag="mx8b")
            nc.vector.max(out=mx8b[:P], in_=x2[:P])
            idx2 = tk_p.tile([128, 8], U32, tag="idx2")
            nc.vector.max_index(idx2[:P], mx8b[:P], x2[:P])
            idxf = tk_p.tile([128, U], F32, tag="idxf")
            nc.vector.tensor_copy(idxf[:P, 0:8], idx1[:P])
            nc.vector.tensor_copy(idxf[:P, 8:U], idx2[:P, 0:2])
            nc.sync.dma_start(idx_d[c * P:(c + 1) * P], idxf[:P])

            # gather offsets: row = (b*H + h)*L + idx  (pair-major shard rows)
            rowb_i = tk_p.tile([128, 1], I32, tag="rowbi")
            nc.gpsimd.iota(rowb_i[:P], pattern=[[0, 1]], base=c * P * L,
                           channel_multiplier=L)
            rowb_f = tk_p.tile([128, 1], F32, tag="rowbf")
            nc.vector.tensor_copy(rowb_f[:P], rowb_i[:P])
            offf = tk_p.tile([128, U], F32, tag="offf")
            nc.vector.tensor_scalar_add(offf[:P], idxf[:P], rowb_f[:P, 0:1])
            offi = tk_p.tile([128, U], I32, tag="offi")
            nc.vector.tensor_copy(offi[:P], offf[:P])

            gath = gath_p.tile([128, U * D], F32, tag="gath")
            for j in range(U):
                nc.gpsimd.indirect_dma_start(
                    out=gath[:P, j * D:(j + 1) * D],
                    out_offset=None,
                    in_=qg_d[:],
                    in_offset=IndirectOffsetOnAxis(ap=offi[:P, j:j + 1], axis=0),
                )
            # Q_red^T: per j, [P, 64] -> [64, P]; assembled [64, U*128]
            qredT = qrt_p.tile([D, U * 128], F32, tag="qredT")
            for j in range(U):
                tp = grp_pp.tile([D, 128], F32, tag="grp")
                nc.tensor.transpose(tp[:, :P], gath[:P, j * D:(j + 1) * D],
                                    ident[:P, :P])
                nc.scalar.copy(qredT[:, j * 128:j * 128 + P], tp[:, :P])
            qredTv = qredT[:].rearrange("d (j c) -> d c j", c=128)

            # ---- softmax / upd over groups of pairs ----
            p0 = 0
            while p0 < P:
                gn = min(GMAX, P - p0)
                rows = gn * U
                r0 = (c * P + p0) * U

                sct = grp_pp.tile([L, U * GMAX], F32, tag="grp")
                for g in range(gn):
                    p = p0 + g
                    bl, h = divmod(p, H)
                    nc.tensor.matmul(
                        sct[:, g * U:(g + 1) * U],
                        lhsT=kts[bl][:, h * L:(h + 1) * L],
                        rhs=qredTv[:, p, :],
                        start=True, stop=True)
                tmpT = sm_p.tile([L, U * GMAX], F32, tag="tmpT")
                nc.vector.tensor_add(tmpT[:, :rows], sct[:, :rows],
                                     rpbt[:, :rows])
                str_ps = grp_pp.tile([U * GMAX, L], F32, tag="grp")
                nc.tensor.transpose(str_ps[:rows, :], tmpT[:, :rows],
                                    ident[:L, :L])
                mxg = sm_p.tile([U * GMAX, 1], F32, tag="mxg")
                nc.vector.reduce_max(mxg[:rows], str_ps[:rows], axis=AX.X)
                nmx = sm_p.tile([U * GMAX, 1], F32, tag="nmx")
                nc.vector.tensor_scalar_mul(nmx[:rows], mxg[:rows], -0.125)
                attne = sm_p.tile([U * GMAX, L], F32, tag="attne")
                ssum = sm_p.tile([U * GMAX, 1], F32, tag="ssum")
                nc.scalar.activation(attne[:rows], str_ps[:rows], ACTF.Exp,
                                     bias=nmx[:rows, 0:1], scale=0.125,
                                     accum_out=ssum[:rows, 0:1])
                rinv = sm_p.tile([U * GMAX, 1], F32, tag="rinv")
                nc.vector.reciprocal(rinv[:rows], ssum[:rows])
                attno = sm_p.tile([U * GMAX, L], F32, tag="attno")
                nc.vector.tensor_scalar_mul(attno[:rows], attne[:rows],
                                            rinv[:rows, 0:1])
                nc.sync.dma_start(attn_d[r0:r0 + rows], attno[:rows])

                atT_ps = grp_pp.tile([L, U * GMAX], F32, tag="grp")
                nc.tensor.transpose(atT_ps[:, :rows], attne[:rows, :],
                                    ident[:rows, :rows])
                atT = sm_p.tile([L, U * GMAX], F32, tag="atT")
                nc.scalar.copy(atT[:, :rows], atT_ps[:, :rows])

                updT_ps = grp_pp.tile([D, U * GMAX], F32, tag="grp")
                for g in range(gn):
                    p = p0 + g
                    bl, h = divmod(p, H)
                    nc.tensor.matmul(
                        updT_ps[:, g * U:(g + 1) * U],
                        lhsT=vbs[bl][:, h * D:(h + 1) * D],
                        rhs=atT[:, g * U:(g + 1) * U],
                        start=True, stop=True)
                updT = sm_p.tile([D, U * GMAX], F32, tag="updT")
                nc.scalar.copy(updT[:, :rows], updT_ps[:, :rows])
                upd_ps = grp_pp.tile([U * GMAX, D], F32, tag="grp")
                nc.tensor.transpose(upd_ps[:rows, :], updT[:, :rows],
                                    ident[:D, :D])
                updsb = sm_p.tile([U * GMAX, D], F32, tag="updsb")
                nc.vector.tensor_scalar_mul(updsb[:rows], upd_ps[:rows],
                                            rinv[:rows, 0:1])
                nc.sync.dma_start(upd_d[r0:r0 + rows], updsb[:rows])
                p0 += gn

    nc.compile()
    return nc


_PROG_CACHE = {}


def _get_prog(bs):
    if bs not in _PROG_CACHE:
        _PROG_CACHE[bs] = build_program(bs)
    return _PROG_CACHE[bs]


def make_in_maps(q, k, v, bt, ncores):
    """Host-side layout prep + sharding. Returns list of per-core input dicts."""
    b_tot = q.shape[0]
    bs = b_tot // ncores
    qt = np.ascontiguousarray(q.transpose(0, 3, 2, 1)).reshape(b_tot, D, H * L)
    ktf = np.ascontiguousarray(k.transpose(0, 3, 2, 1))        # [B, D, H, L]
    keven = ktf[:, :, :, 0::2]                                  # [B, D, H, 41]
    ksum = -keven.sum(-1, keepdims=True) / np.float32(L)
    ktx = np.ascontiguousarray(
        np.concatenate([keven, ksum], -1)).reshape(b_tot, D, H * SX)
    kt = ktf.reshape(b_tot, D, H * L)
    vr = v.reshape(b_tot, L, HD)
    qg = np.ascontiguousarray(q.transpose(0, 2, 1, 3)).reshape(b_tot * H * L, D)

    rel = _rel_pos_index(WS)
    rpb = bt[rel.ravel(), 0].reshape(L * L // L, L)[:U, :]      # [10, 81]
    rpbt = np.ascontiguousarray(np.tile(rpb.T, (1, GMAX)))      # [81, 120]
    ltri = np.triu(np.ones((L, L), np.float32))                 # L[k, t] = k<=t

    in_maps = []
    for c in range(ncores):
        sl = slice(c * bs, (c + 1) * bs)
        in_maps.append({
            "qt_in": qt[sl],
            "ktx_in": ktx[sl],
            "kt_in": kt[sl],
            "v_in": np.ascontiguousarray(vr[sl]),
            "qg_in": qg[c * bs * H * L:(c + 1) * bs * H * L],
            "rpbt_in": rpbt,
            "ltri_in": ltri,
        })
    return in_maps, bs


def assemble(results, ncores, bs):
    """Host-side unsharding + scatter-merge of the top-10 updated rows."""
    b_tot = ncores * bs
    ctx_full = np.empty((b_tot, L, H, D), np.float32)
    attn_full = np.empty((b_tot, H, U, L), np.float32)
    bi = np.arange(bs)[:, None, None]
    hi = np.arange(H)[None, :, None]
    for c in range(ncores):
        r = results[c]
        cs = np.array(r["ctx_out"]).reshape(bs, L, H, D)
        at = np.asarray(r["attn_out"]).reshape(bs, H, U, L)
        ud = np.asarray(r["upd_out"]).reshape(bs, H, U, D)
        ix = np.rint(r["idx_out"]).astype(np.int64).reshape(bs, H, U)
        cs[bi, ix, hi] = ud
        ctx_full[c * bs:(c + 1) * bs] = cs
        attn_full[c * bs:(c + 1) * bs] = at
    return ctx_full, attn_full


def kernel(queries, keys, values, bias_table, attn_mask=None, _trace=False):
    q = np.ascontiguousarray(np.asarray(queries, dtype=np.float32))
    k = np.ascontiguousarray(np.asarray(keys, dtype=np.float32))
    v = np.ascontiguousarray(np.asarray(values, dtype=np.float32))
    bt = np.asarray(bias_table, dtype=np.float32)

    in_maps, bs = make_in_maps(q, k, v, bt, NCORES)
    nc = _get_prog(bs)
    res = run_bass_kernel_spmd(nc, in_maps, list(range(NCORES)), trace=_trace)
    out = assemble(res.results, NCORES, bs)
    if _trace:
        return out, res
    return out


# revision 14
# speedup vs baseline: 1.2521x; 1.2521x over previous
"""ProbAttention (sparse attention) Trainium2 kernel.

Reference computation per (b, h):
    QK_s = Q @ K_even^T                       [81, 41]   (even-indexed keys)
    M    = QK_s.max(-1) - QK_s.sum(-1)/81     [81]
    top10 = top_k(M, 10) indices              (descending M)
    scores = (Q[top10] @ K^T + rpb[:10]) / 8  [10, 81]
    attn = softmax(scores, -1)                [10, 81]
    ctx  = cumsum(V, seq); ctx[top10] = attn @ V
Outputs: (ctx swapped to [B, L, H, D], attn [B, H, 10, 81])

Device strategy (pure data parallel over B, 128 window-batches per core):
  - Host pre-transposes Q, K into [d, l] layouts, parity-stacked so head h
    lives at SBUF partitions (h%2)*64..+64: consecutive per-head matmuls
    target different PE row-groups, letting LDWEIGHTS overlap in-flight
    matmuls and pairs of matmuls run concurrently in the array.
  - Phase 1: per (b,h) matmul Qt^T @ [K_even | -Ksum/81] -> [81, 42] PSUM;
    grouped free-dim reduce gives M for 8 heads at once; per 16-b chunk the
    [81, 128] M matrix is PE-transposed and top-10 found with the DVE
    max8/max_index/match_replace ops (two passes).
  - Top-10 query rows are fetched with an indirect (gather) DMA and
    PE-transposed back into [d, u] operand layout (both partition halves).
  - scores are computed transposed ([81, u] per pair, batched 12 pairs per
    PSUM bank along the free dim), bias-added, PE-transposed to [u*12, 81]
    and softmaxed in batch (ACT exp with fused scale/bias/accum-sum).
  - upd = attn_norm @ V computed transposed ([64, u*12] batched).
  - ctx cumsum is one lower-triangular matmul per b: L^T(tri) @ V -> [81, 512].
  - The 10-row scatter of upd into ctx is done on the host during unsharding
    (device returns dense ctx, upd rows, and the top-10 indices).
"""

import sys

sys.path.insert(0, "/opt/trn_rl_repo")

from contextlib import ExitStack

import numpy as np

from concourse import bacc, bass, mybir, tile
from concourse.bass import IndirectOffsetOnAxis
from concourse.bass_utils import run_bass_kernel_spmd
from concourse.masks import make_identity

B, L, H, D = 1024, 81, 8, 64
NCORES = 8
WS = 9
U = 10          # n_top queries
S = 41          # sampled (even) keys
SX = S + 1      # + folded -sum/81 column
HD = H * D      # 512
HG = H // 2     # head-groups per partition half
GMAX = 12       # pairs per softmax group (12*10=120 <= 128 partitions)

F32 = mybir.dt.float32
I32 = mybir.dt.int32
U32 = mybir.dt.uint32
AX = mybir.AxisListType
ALU = mybir.AluOpType
ACTF = mybir.ActivationFunctionType


def _rel_pos_index(ws):
    coords = np.stack(np.meshgrid(np.arange(ws), np.arange(ws), indexing="ij"))
    cf = coords.reshape(2, -1)
    rel = (cf[:, :, None] - cf[:, None, :]).transpose(1, 2, 0)
    rel[..., 0] += ws - 1
    rel[..., 1] += ws - 1
    rel[..., 0] *= 2 * ws - 1
    return rel.sum(-1)


def build_program(bs):
    """Build the SPMD Bass program for a per-core shard of `bs` batches."""
    ch_b = min(16, bs)          # batches per chunk
    assert bs % ch_b == 0 and ch_b % 2 == 0
    nch = bs // ch_b
    P = ch_b * H                # (b, h) pairs per chunk (<= 128)

    nc = bacc.Bacc("TRN2", target_bir_lowering=False, debug=False,
                   num_devices=NCORES)

    # parity-stacked: partition = (h%2)*64 + d, free col = (h//2)*W + c
    qt_d = nc.dram_tensor("qt_in", [bs, 128, HG * L], F32, kind="ExternalInput").ap()
    ktx_d = nc.dram_tensor("ktx_in", [bs, 128, HG * SX], F32, kind="ExternalInput").ap()
    kt_d = nc.dram_tensor("kt_in", [bs, 128, HG * L], F32, kind="ExternalInput").ap()
    v_d = nc.dram_tensor("v_in", [bs, L, HD], F32, kind="ExternalInput").ap()
    qg_d = nc.dram_tensor("qg_in", [bs * H * L, D], F32, kind="ExternalInput").ap()
    rpbt_d = nc.dram_tensor("rpbt_in", [L, U * GMAX], F32, kind="ExternalInput").ap()
    ltri_d = nc.dram_tensor("ltri_in", [L, L], F32, kind="ExternalInput").ap()

    ctx_d = nc.dram_tensor("ctx_out", [bs * L, HD], F32, kind="ExternalOutput").ap()
    attn_d = nc.dram_tensor("attn_out", [bs * H * U, L], F32, kind="ExternalOutput").ap()
    upd_d = nc.dram_tensor("upd_out", [bs * H * U, D], F32, kind="ExternalOutput").ap()
    idx_d = nc.dram_tensor("idx_out", [bs * H, U], F32, kind="ExternalOutput").ap()

    with tile.TileContext(nc) as tc, ExitStack() as ctx:
        pool = lambda name, bufs, space="SBUF": ctx.enter_context(
            tc.tile_pool(name=name, bufs=bufs, space=space))

        const_p = pool("const", 1)
        qt_p = pool("qt", 3)
        ktx_p = pool("ktx", 3)
        kt_p = pool("kt", ch_b // 2 + 2)
        v_p = pool("v", ch_b // 2 + 2)
        mx_p = pool("mx", 3)
        mc_p = pool("mc", 2)
        tk_p = pool("tk", 2)
        gath_p = pool("gath", 2)
        qrt_p = pool("qrt", 2)
        sm_p = pool("sm", 4)
        ctxsb_p = pool("ctxsb", 3)

        qks_pp = pool("qks_pp", 1, "PSUM")
        cum_pp = pool("cum_pp", 2, "PSUM")
        grp_pp = pool("grp_pp", 4, "PSUM")

        ident = const_p.tile([128, 128], F32, tag="ident")
        make_identity(nc, ident[:])
        rpbt = const_p.tile([L, U * GMAX], F32, tag="rpbt")
        nc.sync.dma_start(rpbt[:], rpbt_d[:])
        ltri = const_p.tile([L, L], F32, tag="ltri")
        nc.sync.dma_start(ltri[:], ltri_d[:])

        def half(t, par, base, n):
            """Slice operand: partitions par*64..+64, free cols base..base+n."""
            return t[par * 64:(par + 1) * 64, base:base + n]

        for c in range(nch):
            mcols = mc_p.tile([L, 128], F32, tag="mcols")
            kts = []
            vbs = []
            for b2 in range(ch_b // 2):   # two batches per DMA
                b = c * ch_b + 2 * b2
                qt = qt_p.tile([128, 2 * HG * L], F32, tag="qt")
                nc.sync.dma_start(qt[:, 0:HG * L], qt_d[b])
                nc.sync.dma_start(qt[:, HG * L:], qt_d[b + 1])
                ktx = ktx_p.tile([128, 2 * HG * SX], F32, tag="ktx")
                nc.sync.dma_start(ktx[:, 0:HG * SX], ktx_d[b])
                nc.sync.dma_start(ktx[:, HG * SX:], ktx_d[b + 1])
                kt = kt_p.tile([128, 2 * HG * L], F32, tag="kt")
                nc.sync.dma_start(kt[:, 0:HG * L], kt_d[b])
                nc.sync.dma_start(kt[:, HG * L:], kt_d[b + 1])
                vb = v_p.tile([L, 2 * HD], F32, tag="v")
                nc.sync.dma_start(vb[:, 0:HD], v_d[b])
                nc.sync.dma_start(vb[:, HD:], v_d[b + 1])
                kts.append(kt)
                vbs.append(vb)

                for bi in range(2):
                    bl = 2 * b2 + bi
                    # Phase 1: QK over sampled keys (+ -sum/81 in col 41)
                    # One PSUM tile per operand-partition parity: the PE
                    # rejects mixed tile_position writes into one bank.
                    qks = [qks_pp.tile([L, HG * SX], F32, tag="qks%d" % t,
                                       name="qks%d" % t)
                           for t in range(2)]
                    for h in range(H):
                        par, hg = h % 2, h // 2
                        nc.tensor.matmul(
                            qks[par][:, hg * SX:(hg + 1) * SX],
                            lhsT=half(qt, par, bi * HG * L + hg * L, L),
                            rhs=half(ktx, par, bi * HG * SX + hg * SX, SX),
                            start=True, stop=True)
                    mcv = mcols[:, bl * H:(bl + 1) * H].rearrange(
                        "p (g t) -> p t g", t=2)
                    for par in range(2):
                        qksv = qks[par][:].rearrange("p (h s) -> p h s", s=SX)
                        mxt = mx_p.tile([L, HG], F32, tag="mxt%d" % par)
                        nc.vector.reduce_max(mxt[:], qksv[:, :, 0:S], axis=AX.X)
                        nc.vector.tensor_add(mcv[:, par], mxt[:], qksv[:, :, S])

                    # causal cumsum of V via lower-tri matmul
                    cum = cum_pp.tile([L, HD], F32, tag="cum")
                    nc.tensor.matmul(
                        cum[:], lhsT=ltri[:], rhs=vb[:, bi * HD:(bi + 1) * HD],
                        start=True, stop=True)
                    ctxsb = ctxsb_p.tile([L, HD], F32, tag="ctxsb")
                    nc.scalar.copy(ctxsb[:], cum[:])
                    nc.sync.dma_start(
                        ctx_d[(c * ch_b + bl) * L:(c * ch_b + bl + 1) * L],
                        ctxsb[:])

            # ---- top-10 per pair over the chunk ----
            mt_ps = grp_pp.tile([128, L], F32, tag="grp")
            nc.tensor.transpose(mt_ps[:P, :], mcols[:, :P], ident[:L, :L])
            xsb = tk_p.tile([128, L], F32, tag="xsb")
            nc.vector.tensor_copy(xsb[:P], mt_ps[:P])
            mx8 = tk_p.tile([128, 8], F32, tag="mx8")
            nc.vector.max(out=mx8[:P], in_=xsb[:P])
            idx1 = tk_p.tile([128, 8], U32, tag="idx1")
            nc.vector.max_index(idx1[:P], mx8[:P], xsb[:P])
            x2 = tk_p.tile([128, L], F32, tag="x2")
            nc.vector.match_replace(out=x2[:P], in_to_replace=mx8[:P],
                                    in_values=xsb[:P], imm_value=-1e30)
            mx8b = tk_p.tile([128, 8], F32, tag="mx8b")
            nc.vector.max(out=mx8b[:P], in_=x2[:P])
            idx2 = tk_p.tile([128, 8], U32, tag="idx2")
            nc.vector.max_index(idx2[:P], mx8b[:P], x2[:P])
            idxf = tk_p.tile([128, U], F32, tag="idxf")
            nc.vector.tensor_copy(idxf[:P, 0:8], idx1[:P])
            nc.vector.tensor_copy(idxf[:P, 8:U], idx2[:P, 0:2])
            nc.sync.dma_start(idx_d[c * P:(c + 1) * P], idxf[:P])

            # gather offsets: row = (b*H + h)*L + idx  (pair-major shard rows)
            rowb_i = tk_p.tile([128, 1], I32, tag="rowbi")
            nc.gpsimd.iota(rowb_i[:P], pattern=[[0, 1]], base=c * P * L,
                           channel_multiplier=L)
            rowb_f = tk_p.tile([128, 1], F32, tag="rowbf")
            nc.vector.tensor_copy(rowb_f[:P], rowb_i[:P])
            offf = tk_p.tile([128, U], F32, tag="offf")
            nc.vector.tensor_scalar_add(offf[:P], idxf[:P], rowb_f[:P, 0:1])
            offi = tk_p.tile([128, U], I32, tag="offi")
            nc.vector.tensor_copy(offi[:P], offf[:P])

            gath = gath_p.tile([128, U * D], F32, tag="gath")
            for j in range(U):
                nc.gpsimd.indirect_dma_start(
                    out=gath[:P, j * D:(j + 1) * D],
                    out_offset=None,
                    in_=qg_d[:],
                    in_offset=IndirectOffsetOnAxis(ap=offi[:P, j:j + 1], axis=0),
                )
            # Q_red^T: per j, [P, 64] -> [64, P]; duplicated to both halves
            qredT = qrt_p.tile([128, U * 128], F32, tag="qredT")
            for j in range(U):
                tp = grp_pp.tile([D, 128], F32, tag="grp")
                nc.tensor.transpose(tp[:, :P], gath[:P, j * D:(j + 1) * D],
                                    ident[:P, :P])
                nc.scalar.copy(qredT[0:D, j * 128:j * 128 + P], tp[:, :P])
                nc.scalar.copy(qredT[D:2 * D, j * 128:j * 128 + P], tp[:, :P])

            # ---- softmax / upd over groups of pairs ----
            p0 = 0
            while p0 < P:
                gn = min(GMAX, P - p0)
                rows = gn * U
                r0 = (c * P + p0) * U

                sct = [grp_pp.tile([L, U * GMAX // 2], F32, tag="grp",
                                      name="sct%d" % t)
                       for t in range(2)]
                for g in range(gn):
                    p = p0 + g
                    bl, h = divmod(p, H)
                    par, hg = h % 2, h // 2
                    qslice = qredT[par * 64:(par + 1) * 64, :].rearrange(
                        "d (j c) -> d c j", c=128)[:, p, :]
                    nc.tensor.matmul(
                        sct[g % 2][:, (g // 2) * U:(g // 2 + 1) * U],
                        lhsT=half(kts[bl // 2], par,
                                  (bl % 2) * HG * L + hg * L, L),
                        rhs=qslice,
                        start=True, stop=True)
                tmpT = sm_p.tile([L, U * GMAX], F32, tag="tmpT")
                tmv = tmpT[:].rearrange("p (g t u) -> p t g u", t=2, u=U)
                for t in range(2):
                    ng = (gn + 1 - t) // 2
                    nc.vector.tensor_add(
                        tmv[:, t, :ng, :],
                        sct[t][:].rearrange("p (g u) -> p g u", u=U)[:, :ng],
                        rpbt[:].rearrange("p (g u) -> p g u", u=U)[:, :ng])
                str_ps = grp_pp.tile([U * GMAX, L], F32, tag="grp")
                nc.tensor.transpose(str_ps[:rows, :], tmpT[:, :rows],
                                    ident[:L, :L])
                mxg = sm_p.tile([U * GMAX, 1], F32, tag="mxg")
                nc.vector.reduce_max(mxg[:rows], str_ps[:rows], axis=AX.X)
                nmx = sm_p.tile([U * GMAX, 1], F32, tag="nmx")
                nc.vector.tensor_scalar_mul(nmx[:rows], mxg[:rows], -0.125)
                attne = sm_p.tile([U * GMAX, L], F32, tag="attne")
                ssum = sm_p.tile([U * GMAX, 1], F32, tag="ssum")
                nc.scalar.activation(attne[:rows], str_ps[:rows], ACTF.Exp,
                                     bias=nmx[:rows, 0:1], scale=0.125,
                                     accum_out=ssum[:rows, 0:1])
                rinv = sm_p.tile([U * GMAX, 1], F32, tag="rinv")
                nc.vector.reciprocal(rinv[:rows], ssum[:rows])
                attno = sm_p.tile([U * GMAX, L], F32, tag="attno")
                nc.vector.tensor_scalar_mul(attno[:rows], attne[:rows],
                                            rinv[:rows, 0:1])
                nc.sync.dma_start(attn_d[r0:r0 + rows], attno[:rows])

                atT_ps = grp_pp.tile([L, U * GMAX], F32, tag="grp")
                nc.tensor.transpose(atT_ps[:, :rows], attno[:rows, :],
                                    ident[:rows, :rows])
                atT = sm_p.tile([L, U * GMAX], F32, tag="atT")
                nc.scalar.copy(atT[:, :rows], atT_ps[:, :rows])

                updT_ps = grp_pp.tile([D, U * GMAX], F32, tag="grp")
                for g in range(gn):
                    p = p0 + g
                    bl, h = divmod(p, H)
                    nc.tensor.matmul(
                        updT_ps[:, g * U:(g + 1) * U],
                        lhsT=vbs[bl // 2][:, (bl % 2) * HD + h * D:
                                          (bl % 2) * HD + (h + 1) * D],
                        rhs=atT[:, g * U:(g + 1) * U],
                        start=True, stop=True)
                updT = sm_p.tile([D, U * GMAX], F32, tag="updT")
                nc.scalar.copy(updT[:, :rows], updT_ps[:, :rows])
                upd_ps = grp_pp.tile([U * GMAX, D], F32, tag="grp")
                nc.tensor.transpose(upd_ps[:rows, :], updT[:, :rows],
                                    ident[:D, :D])
                updsb = sm_p.tile([U * GMAX, D], F32, tag="updsb")
                nc.vector.tensor_copy(updsb[:rows], upd_ps[:rows])
                nc.sync.dma_start(upd_d[r0:r0 + rows], updsb[:rows])
                p0 += gn

    nc.compile()
    return nc


_PROG_CACHE = {}


def _get_prog(bs):
    if bs not in _PROG_CACHE:
        _PROG_CACHE[bs] = build_program(bs)
    return _PROG_CACHE[bs]


def _parity_stack(a):
    """[B, d, h, w] -> [B, 128, (h//2)*w] with partition (h%2)*64+d."""
    b, d, h, w = a.shape
    return np.ascontiguousarray(
        a.transpose(0, 2, 1, 3).reshape(b, h // 2, 2, d, w)
        .transpose(0, 2, 3, 1, 4)).reshape(b, 2 * d, (h // 2) * w)


def make_in_maps(q, k, v, bt, ncores):
    """Host-side layout prep + sharding. Returns list of per-core input dicts."""
    b_tot = q.shape[0]
    bs = b_tot // ncores
    qtf = np.ascontiguousarray(q.transpose(0, 3, 2, 1))        # [B, D, H, L]
    ktf = np.ascontiguousarray(k.transpose(0, 3, 2, 1))        # [B, D, H, L]
    keven = ktf[:, :, :, 0::2]                                  # [B, D, H, 41]
    ksum = -keven.sum(-1, keepdims=True) / np.float32(L)
    ktxf = np.concatenate([keven, ksum], -1)                    # [B, D, H, 42]
    qt = _parity_stack(qtf)                                     # [B, 128, 324]
    kt = _parity_stack(ktf)
    ktx = _parity_stack(ktxf)                                   # [B, 128, 168]
    vr = v.reshape(b_tot, L, HD)
    qg = np.ascontiguousarray(q.transpose(0, 2, 1, 3)).reshape(b_tot * H * L, D)

    rel = _rel_pos_index(WS)
    rpb = bt[rel.ravel(), 0].reshape(L, L)[:U, :]               # [10, 81]
    rpbt = np.ascontiguousarray(np.tile(rpb.T, (1, GMAX)))      # [81, 120]
    ltri = np.triu(np.ones((L, L), np.float32))                 # L[k, t] = k<=t

    in_maps = []
    for c in range(ncores):
        sl = slice(c * bs, (c + 1) * bs)
        in_maps.append({
            "qt_in": qt[sl],
            "ktx_in": ktx[sl],
            "kt_in": kt[sl],
            "v_in": np.ascontiguousarray(vr[sl]),
            "qg_in": qg[c * bs * H * L:(c + 1) * bs * H * L],
            "rpbt_in": rpbt,
            "ltri_in": ltri,
        })
    return in_maps, bs


def assemble(results, ncores, bs):
    """Host-side unsharding + scatter-merge of the top-10 updated rows."""
    b_tot = ncores * bs
    ctx_full = np.empty((b_tot, L, H, D), np.float32)
    attn_full = np.empty((b_tot, H, U, L), np.float32)
    bi = np.arange(bs)[:, None, None]
    hi = np.arange(H)[None, :, None]
    for c in range(ncores):
        r = results[c]
        cs = np.array(r["ctx_out"]).reshape(bs, L, H, D)
        at = np.asarray(r["attn_out"]).reshape(bs, H, U, L)
        ud = np.asarray(r["upd_out"]).reshape(bs, H, U, D)
        ix = np.rint(r["idx_out"]).astype(np.int64).reshape(bs, H, U)
        cs[bi, ix, hi] = ud
        ctx_full[c * bs:(c + 1) * bs] = cs
        attn_full[c * bs:(c + 1) * bs] = at
    return ctx_full, attn_full


def kernel(queries, keys, values, bias_table, attn_mask=None, _trace=False):
    q = np.ascontiguousarray(np.asarray(queries, dtype=np.float32))
    k = np.ascontiguousarray(np.asarray(keys, dtype=np.float32))
    v = np.ascontiguousarray(np.asarray(values, dtype=np.float32))
    bt = np.asarray(bias_table, dtype=np.float32)

    in_maps, bs = make_in_maps(q, k, v, bt, NCORES)
    nc = _get_prog(bs)
    res = run_bass_kernel_spmd(nc, in_maps, list(range(NCORES)), trace=_trace)
    out = assemble(res.results, NCORES, bs)
    if _trace:
        return out, res
    return out


# revision 16
# speedup vs baseline: 1.3863x; 1.1072x over previous
"""ProbAttention (sparse attention) Trainium2 kernel.

Reference computation per (b, h):
    QK_s = Q @ K_even^T                       [81, 41]   (even-indexed keys)
    M    = QK_s.max(-1) - QK_s.sum(-1)/81     [81]
    top10 = top_k(M, 10) indices              (descending M)
    scores = (Q[top10] @ K^T + rpb[:10]) / 8  [10, 81]
    attn = softmax(scores, -1)                [10, 81]
    ctx  = cumsum(V, seq); ctx[top10] = attn @ V
Outputs: (ctx swapped to [B, L, H, D], attn [B, H, 10, 81])

Device strategy (pure data parallel over B, 128 window-batches per core).
The PE issue rate (~66 ns/instruction) dominates at these tiny matmul
sizes, so everything is packed two (b,h) pairs per matmul:
  - Q/K are host-transposed to [d, l] and head-pair stacked: head 2a at
    partitions 0..63, head 2a+1 at 64..127 of free column block a. A single
    [128, 81] lhsT then carries both heads.
  - QK_s: rhs is a host-built block-diagonal [128, 84] ([K_even|-sum/81]
    for head 2a in the top-left, head 2a+1 bottom-right) -> one matmul
    yields both heads' [81, 42] score blocks.
  - scores^T: lhsT = stacked K pair, rhs = device-built block-diagonal
    Q_red^T [128, 20] (gathered top-10 query rows, PE-transposed into
    parity halves of a zero-initialized tile) -> [81, 20] for two pairs,
    batched 12 pairs per PSUM bank; bias-added, PE-transposed and
    softmaxed in batch (ACT exp with fused scale/bias/accum-sum).
  - upd^T: lhsT = two heads' V [81, 128], rhs = normalized attn^T [81, 20];
    the off-diagonal [64, 10] quadrants of the [128, 20] output are unused
    garbage, diagonal quadrants are the two pairs' upd^T.
  - ctx cumsum: one lower-triangular matmul per b (L^T(tri) @ V).
  - top-10 per pair via DVE max8/max_index/match_replace (two passes) on a
    PE-transposed [128pairs, 81] M matrix, per 16-b chunk.
  - attn/upd leave the device transposed, one bulk DMA per chunk; the host
    un-transposes during unsharding and scatters upd rows into ctx.
Inputs are shipped partition-major ([128, bs*w]) so multi-batch loads are
a single large-descriptor DMA.
"""

import sys

sys.path.insert(0, "/opt/trn_rl_repo")

from contextlib import ExitStack

import numpy as np

from concourse import bacc, bass, mybir, tile
from concourse.bass import IndirectOffsetOnAxis
from concourse.bass_utils import run_bass_kernel_spmd
from concourse.masks import make_identity

B, L, H, D = 1024, 81, 8, 64
NCORES = 8
WS = 9
U = 10          # n_top queries
S = 41          # sampled (even) keys
SX = S + 1      # + folded -sum/81 column
HD = H * D      # 512
HG = H // 2     # head-pair blocks
GMAX = 12       # pairs per softmax group (12*10=120 <= 128 partitions)

F32 = mybir.dt.float32
I32 = mybir.dt.int32
U32 = mybir.dt.uint32
AX = mybir.AxisListType
ACTF = mybir.ActivationFunctionType


def _rel_pos_index(ws):
    coords = np.stack(np.meshgrid(np.arange(ws), np.arange(ws), indexing="ij"))
    cf = coords.reshape(2, -1)
    rel = (cf[:, :, None] - cf[:, None, :]).transpose(1, 2, 0)
    rel[..., 0] += ws - 1
    rel[..., 1] += ws - 1
    rel[..., 0] *= 2 * ws - 1
    return rel.sum(-1)


def build_program(bs):
    """Build the SPMD Bass program for a per-core shard of `bs` batches."""
    ch_b = min(16, bs)          # batches per chunk
    assert bs % ch_b == 0 and ch_b % 4 == 0
    nch = bs // ch_b
    P = ch_b * H                # (b, h) pairs per chunk (<= 128)
    NB = 4                      # batches per load DMA

    nc = bacc.Bacc("TRN2", target_bir_lowering=False, debug=False,
                   num_devices=NCORES)

    qt_d = nc.dram_tensor("qt_in", [128, bs * HG * L], F32, kind="ExternalInput").ap()
    ktx_d = nc.dram_tensor("ktx_in", [128, bs * HG * 2 * SX], F32,
                           kind="ExternalInput").ap()
    kt_d = nc.dram_tensor("kt_in", [128, bs * HG * L], F32, kind="ExternalInput").ap()
    v_d = nc.dram_tensor("v_in", [L, bs * HD], F32, kind="ExternalInput").ap()
    qg_d = nc.dram_tensor("qg_in", [bs * H * L, D], F32, kind="ExternalInput").ap()
    rpbt_d = nc.dram_tensor("rpbt_in", [L, U * GMAX], F32, kind="ExternalInput").ap()
    ltri_d = nc.dram_tensor("ltri_in", [L, L], F32, kind="ExternalInput").ap()

    ctx_d = nc.dram_tensor("ctx_out", [bs * L, HD], F32, kind="ExternalOutput").ap()
    attn_d = nc.dram_tensor("attn_out", [nch, L, P * U], F32,
                            kind="ExternalOutput").ap()
    upd_d = nc.dram_tensor("upd_out", [nch, D, P * U], F32,
                           kind="ExternalOutput").ap()
    idx_d = nc.dram_tensor("idx_out", [bs * H, U], F32, kind="ExternalOutput").ap()

    W_Q = HG * L                # 324 free cols per batch (qt/kt)
    W_X = HG * 2 * SX           # 336 free cols per batch (ktx, block-diag)

    with tile.TileContext(nc) as tc, ExitStack() as ctx:
        pool = lambda name, bufs, space="SBUF": ctx.enter_context(
            tc.tile_pool(name=name, bufs=bufs, space=space))

        const_p = pool("const", 1)
        qt_p = pool("qt", 3)
        ktx_p = pool("ktx", 3)
        kt_p = pool("kt", ch_b // 4 + 2)
        v_p = pool("v", ch_b // 4 + 2)
        mx_p = pool("mx", 3)
        mc_p = pool("mc", 2)
        tk_p = pool("tk", 2)
        gath_p = pool("gath", 2)
        qrt_p = pool("qrt", 1)
        sm_p = pool("sm", 4)
        ctxsb_p = pool("ctxsb", 3)
        och_p = pool("och", 2)

        qks_pp = pool("qks_pp", 2, "PSUM")
        cum_pp = pool("cum_pp", 2, "PSUM")
        grp_pp = pool("grp_pp", 4, "PSUM")

        ident = const_p.tile([128, 128], F32, tag="ident")
        make_identity(nc, ident[:])
        rpbt = const_p.tile([L, U * GMAX], F32, tag="rpbt")
        nc.sync.dma_start(rpbt[:], rpbt_d[:])
        ltri = const_p.tile([L, L], F32, tag="ltri")
        nc.sync.dma_start(ltri[:], ltri_d[:])

        # Persistent block-diagonal Q_red^T tiles (zero halves live forever).
        q2t = [qrt_p.tile([128, U * 128], F32, tag="qrt%d" % i,
                          name="qrt%d" % i) for i in range(2)]
        for t in q2t:
            nc.gpsimd.memset(t[:], 0.0)

        for c in range(nch):
            mcols = mc_p.tile([L, 128], F32, tag="mcols")
            kts = []
            vbs = []
            for b4 in range(ch_b // NB):
                b = c * ch_b + NB * b4
                qt = qt_p.tile([128, NB * W_Q], F32, tag="qt")
                nc.sync.dma_start(qt[:], qt_d[:, b * W_Q:(b + NB) * W_Q])
                ktx = ktx_p.tile([128, NB * W_X], F32, tag="ktx")
                nc.sync.dma_start(ktx[:], ktx_d[:, b * W_X:(b + NB) * W_X])
                kt = kt_p.tile([128, NB * W_Q], F32, tag="kt")
                nc.sync.dma_start(kt[:], kt_d[:, b * W_Q:(b + NB) * W_Q])
                vb = v_p.tile([L, NB * HD], F32, tag="v")
                nc.sync.dma_start(vb[:], v_d[:, b * HD:(b + NB) * HD])
                kts.append(kt)
                vbs.append(vb)

                for bi in range(NB):
                    bl = NB * b4 + bi
                    # Phase 1: one matmul per head pair -> [81, 84]
                    qks = qks_pp.tile([L, H * SX], F32, tag="qks")
                    for a in range(HG):
                        nc.tensor.matmul(
                            qks[:, a * 2 * SX:(a + 1) * 2 * SX],
                            lhsT=qt[:, bi * W_Q + a * L:bi * W_Q + (a + 1) * L],
                            rhs=ktx[:, bi * W_X + a * 2 * SX:
                                    bi * W_X + (a + 1) * 2 * SX],
                            start=True, stop=True)
                    qksv = qks[:].rearrange("p (h s) -> p h s", s=SX)
                    mxt = mx_p.tile([L, H], F32, tag="mxt")
                    nc.vector.reduce_max(mxt[:], qksv[:, :, 0:S], axis=AX.X)
                    nc.vector.tensor_add(
                        mcols[:, bl * H:(bl + 1) * H], mxt[:], qksv[:, :, S])

                    # causal cumsum of V via lower-tri matmul
                    cum = cum_pp.tile([L, HD], F32, tag="cum")
                    nc.tensor.matmul(
                        cum[:], lhsT=ltri[:], rhs=vb[:, bi * HD:(bi + 1) * HD],
                        start=True, stop=True)
                    ctxsb = ctxsb_p.tile([L, HD], F32, tag="ctxsb")
                    nc.scalar.copy(ctxsb[:], cum[:])
                    nc.sync.dma_start(
                        ctx_d[(c * ch_b + bl) * L:(c * ch_b + bl + 1) * L],
                        ctxsb[:])

            # ---- top-10 per pair over the chunk ----
            mt_ps = grp_pp.tile([128, L], F32, tag="grp")
            nc.tensor.transpose(mt_ps[:P, :], mcols[:, :P], ident[:L, :L])
            xsb = tk_p.tile([128, L], F32, tag="xsb")
            nc.vector.tensor_copy(xsb[:P], mt_ps[:P])
            mx8 = tk_p.tile([128, 8], F32, tag="mx8")
            nc.vector.max(out=mx8[:P], in_=xsb[:P])
            idx1 = tk_p.tile([128, 8], U32, tag="idx1")
            nc.vector.max_index(idx1[:P], mx8[:P], xsb[:P])
            x2 = tk_p.tile([128, L], F32, tag="x2")
            nc.vector.match_replace(out=x2[:P], in_to_replace=mx8[:P],
                                    in_values=xsb[:P], imm_value=-1e30)
            mx8b = tk_p.tile([128, 8], F32, tag="mx8b")
            nc.vector.max(out=mx8b[:P], in_=x2[:P])
            idx2 = tk_p.tile([128, 8], U32, tag="idx2")
            nc.vector.max_index(idx2[:P], mx8b[:P], x2[:P])
            idxf = tk_p.tile([128, U], F32, tag="idxf")
            nc.vector.tensor_copy(idxf[:P, 0:8], idx1[:P])
            nc.vector.tensor_copy(idxf[:P, 8:U], idx2[:P, 0:2])
            nc.sync.dma_start(idx_d[c * P:(c + 1) * P], idxf[:P])

            # gather offsets: row = (b*H + h)*L + idx  (pair-major shard rows)
            rowb_i = tk_p.tile([128, 1], I32, tag="rowbi")
            nc.gpsimd.iota(rowb_i[:P], pattern=[[0, 1]], base=c * P * L,
                           channel_multiplier=L)
            rowb_f = tk_p.tile([128, 1], F32, tag="rowbf")
            nc.vector.tensor_copy(rowb_f[:P], rowb_i[:P])
            offf = tk_p.tile([128, U], F32, tag="offf")
            nc.vector.tensor_scalar_add(offf[:P], idxf[:P], rowb_f[:P, 0:1])
            offi = tk_p.tile([128, U], I32, tag="offi")
            nc.vector.tensor_copy(offi[:P], offf[:P])

            gath = gath_p.tile([128, U * D], F32, tag="gath")
            for j in range(U):
                nc.gpsimd.indirect_dma_start(
                    out=gath[:P, j * D:(j + 1) * D],
                    out_offset=None,
                    in_=qg_d[:],
                    in_offset=IndirectOffsetOnAxis(ap=offi[:P, j:j + 1], axis=0),
                )
            # Q_red^T -> block-diagonal tile: pair p's 10 cols at p*10, rows
            # (p%2)*64..+64; the other half stays zero.
            qredT = q2t[c % 2]
            qrv = qredT[:].rearrange("p (a c) -> p a c", c=2 * U)
            for j in range(U):
                tp = grp_pp.tile([D, 128], F32, tag="grp")
                nc.tensor.transpose(tp[:, :P], gath[:P, j * D:(j + 1) * D],
                                    ident[:P, :P])
                tpv = tp[:].rearrange("d (a t) -> d a t", t=2)
                for par in range(2):
                    nc.scalar.copy(
                        qrv[par * D:(par + 1) * D, :P // 2, par * U + j],
                        tpv[:, :P // 2, par])

            # chunk-level transposed outputs (one DMA each per chunk)
            atT_ch = och_p.tile([L, P * U], F32, tag="atT")
            updT_ch = och_p.tile([D, P * U], F32, tag="updT")

            # ---- softmax / upd over groups of pairs ----
            p0 = 0
            while p0 < P:
                gn = min(GMAX, P - p0)
                rows = gn * U

                sct = grp_pp.tile([L, U * GMAX], F32, tag="grp")
                for a in range(gn // 2):
                    p = p0 + 2 * a
                    bl, h = divmod(p, H)
                    m = h // 2
                    nc.tensor.matmul(
                        sct[:, a * 2 * U:(a + 1) * 2 * U],
                        lhsT=kts[bl // NB][:, (bl % NB) * W_Q + m * L:
                                           (bl % NB) * W_Q + (m + 1) * L],
                        rhs=qredT[:, p * U:(p + 2) * U],
                        start=True, stop=True)
                tmpT = sm_p.tile([L, U * GMAX], F32, tag="tmpT")
                nc.vector.tensor_add(tmpT[:, :rows], sct[:, :rows],
                                     rpbt[:, :rows])
                str_ps = grp_pp.tile([U * GMAX, L], F32, tag="grp")
                nc.tensor.transpose(str_ps[:rows, :], tmpT[:, :rows],
                                    ident[:L, :L])
                mxg = sm_p.tile([U * GMAX, 1], F32, tag="mxg")
                nc.vector.reduce_max(mxg[:rows], str_ps[:rows], axis=AX.X)
                nmx = sm_p.tile([U * GMAX, 1], F32, tag="nmx")
                nc.vector.tensor_scalar_mul(nmx[:rows], mxg[:rows], -0.125)
                attne = sm_p.tile([U * GMAX, L], F32, tag="attne")
                ssum = sm_p.tile([U * GMAX, 1], F32, tag="ssum")
                nc.scalar.activation(attne[:rows], str_ps[:rows], ACTF.Exp,
                                     bias=nmx[:rows, 0:1], scale=0.125,
                                     accum_out=ssum[:rows, 0:1])
                rinv = sm_p.tile([U * GMAX, 1], F32, tag="rinv")
                nc.vector.reciprocal(rinv[:rows], ssum[:rows])
                attno = sm_p.tile([U * GMAX, L], F32, tag="attno")
                nc.vector.tensor_scalar_mul(attno[:rows], attne[:rows],
                                            rinv[:rows, 0:1])

                atT_ps = grp_pp.tile([L, U * GMAX], F32, tag="grp")
                nc.tensor.transpose(atT_ps[:, :rows], attno[:rows, :],
                                    ident[:rows, :rows])
                nc.scalar.copy(atT_ch[:, p0 * U:p0 * U + rows],
                               atT_ps[:, :rows])

                updT_ps = grp_pp.tile([128, U * GMAX], F32, tag="grp")
                for a in range(gn // 2):
                    p = p0 + 2 * a
                    bl, h = divmod(p, H)
                    nc.tensor.matmul(
                        updT_ps[:, a * 2 * U:(a + 1) * 2 * U],
                        lhsT=vbs[bl // NB][:, (bl % NB) * HD + h * D:
                                           (bl % NB) * HD + (h + 2) * D],
                        rhs=atT_ch[:, p * U:(p + 2) * U],
                        start=True, stop=True)
                # diagonal quadrants -> updT_ch; off-diagonals are garbage
                upv = updT_ps[:, :rows].rearrange("d (a t u) -> d t a u",
                                                  t=2, u=U)
                ucv = updT_ch[:, p0 * U:p0 * U + rows].rearrange(
                    "d (a t u) -> d t a u", t=2, u=U)
                for par in range(2):
                    nc.scalar.copy(ucv[:, par],
                                   upv[par * D:(par + 1) * D, par])
                p0 += gn

            nc.sync.dma_start(attn_d[c], atT_ch[:])
            nc.sync.dma_start(upd_d[c], updT_ch[:])

    nc.compile()
    return nc


_PROG_CACHE = {}


def _get_prog(bs):
    if bs not in _PROG_CACHE:
        _PROG_CACHE[bs] = build_program(bs)
    return _PROG_CACHE[bs]


def _pair_stack(a):
    """[B, d, h, w] -> [B, 2d, (h//2)*w]: head 2a+par at rows par*d, col a*w."""
    b, d, h, w = a.shape
    return np.ascontiguousarray(
        a.transpose(0, 2, 1, 3).reshape(b, h // 2, 2, d, w)
        .transpose(0, 2, 3, 1, 4)).reshape(b, 2 * d, (h // 2) * w)


def make_in_maps(q, k, v, bt, ncores):
    """Host-side layout prep + sharding. Returns list of per-core input dicts."""
    b_tot = q.shape[0]
    bs = b_tot // ncores
    qtf = np.ascontiguousarray(q.transpose(0, 3, 2, 1))        # [B, D, H, L]
    ktf = np.ascontiguousarray(k.transpose(0, 3, 2, 1))        # [B, D, H, L]
    keven = ktf[:, :, :, 0::2]                                  # [B, D, H, 41]
    ksum = -keven.sum(-1, keepdims=True) / np.float32(L)
    ktxf = np.concatenate([keven, ksum], -1)                    # [B, D, H, 42]
    qt = _pair_stack(qtf)                                       # [B, 128, 324]
    kt = _pair_stack(ktf)
    # block-diagonal [B, 128, HG*2*SX]: head 2a top-left, 2a+1 bottom-right
    ktx = np.zeros((b_tot, 2, D, HG, 2, SX), np.float32)
    ktx[:, 0, :, :, 0, :] = ktxf[:, :, 0::2].transpose(0, 1, 2, 3)
    ktx[:, 1, :, :, 1, :] = ktxf[:, :, 1::2]
    ktx = ktx.reshape(b_tot, 128, HG * 2 * SX)
    vr = v.reshape(b_tot, L, HD)
    qg = np.ascontiguousarray(q.transpose(0, 2, 1, 3)).reshape(b_tot * H * L, D)

    rel = _rel_pos_index(WS)
    rpb = bt[rel.ravel(), 0].reshape(L, L)[:U, :]               # [10, 81]
    rpbt = np.ascontiguousarray(np.tile(rpb.T, (1, GMAX)))      # [81, 120]
    ltri = np.triu(np.ones((L, L), np.float32))                 # L[k, t] = k<=t

    in_maps = []
    for c in range(ncores):
        sl = slice(c * bs, (c + 1) * bs)
        in_maps.append({
            "qt_in": np.ascontiguousarray(
                qt[sl].transpose(1, 0, 2)).reshape(128, bs * HG * L),
            "ktx_in": np.ascontiguousarray(
                ktx[sl].transpose(1, 0, 2)).reshape(128, bs * HG * 2 * SX),
            "kt_in": np.ascontiguousarray(
                kt[sl].transpose(1, 0, 2)).reshape(128, bs * HG * L),
            "v_in": np.ascontiguousarray(
                vr[sl].transpose(1, 0, 2)).reshape(L, bs * HD),
            "qg_in": qg[c * bs * H * L:(c + 1) * bs * H * L],
            "rpbt_in": rpbt,
            "ltri_in": ltri,
        })
    return in_maps, bs


def assemble(results, ncores, bs):
    """Host-side unsharding, un-transposing, and top-10 scatter-merge."""
    b_tot = ncores * bs
    ch_b = min(16, bs)
    nch = bs // ch_b
    P = ch_b * H
    ctx_full = np.empty((b_tot, L, H, D), np.float32)
    attn_full = np.empty((b_tot, H, U, L), np.float32)
    bi = np.arange(bs)[:, None, None]
    hi = np.arange(H)[None, :, None]
    for c in range(ncores):
        r = results[c]
        cs = np.array(r["ctx_out"]).reshape(bs, L, H, D)
        # [nch, L, P, U] -> [nch, P, U, L] -> [bs, H, U, L]
        at = np.asarray(r["attn_out"]).reshape(nch, L, P, U) \
            .transpose(0, 2, 3, 1).reshape(bs, H, U, L)
        ud = np.asarray(r["upd_out"]).reshape(nch, D, P, U) \
            .transpose(0, 2, 3, 1).reshape(bs, H, U, D)
        ix = np.rint(r["idx_out"]).astype(np.int64).reshape(bs, H, U)
        cs[bi, ix, hi] = ud
        ctx_full[c * bs:(c + 1) * bs] = cs
        attn_full[c * bs:(c + 1) * bs] = at
    return ctx_full, attn_full


def kernel(queries, keys, values, bias_table, attn_mask=None, _trace=False):
    q = np.ascontiguousarray(np.asarray(queries, dtype=np.float32))
    k = np.ascontiguousarray(np.asarray(keys, dtype=np.float32))
    v = np.ascontiguousarray(np.asarray(values, dtype=np.float32))
    bt = np.asarray(bias_table, dtype=np.float32)

    in_maps, bs = make_in_maps(q, k, v, bt, NCORES)
    nc = _get_prog(bs)
    res = run_bass_kernel_spmd(nc, in_maps, list(range(NCORES)), trace=_trace)
    out = assemble(res.results, NCORES, bs)
    if _trace:
        return out, res
    return out


# revision 18
# speedup vs baseline: 1.4039x; 1.0127x over previous
"""ProbAttention (sparse attention) Trainium2 kernel.

Reference computation per (b, h):
    QK_s = Q @ K_even^T                       [81, 41]   (even-indexed keys)
    M    = QK_s.max(-1) - QK_s.sum(-1)/81     [81]
    top10 = top_k(M, 10) indices              (descending M)
    scores = (Q[top10] @ K^T + rpb[:10]) / 8  [10, 81]
    attn = softmax(scores, -1)                [10, 81]
    ctx  = cumsum(V, seq); ctx[top10] = attn @ V
Outputs: (ctx swapped to [B, L, H, D], attn [B, H, 10, 81])

Device strategy (pure data parallel over B, 128 window-batches per core).
The PE issue rate (~66 ns/instruction) dominates at these tiny matmul
sizes, so everything is packed two (b,h) pairs per matmul:
  - Q/K are host-transposed to [d, l] and head-pair stacked: head 2a at
    partitions 0..63, head 2a+1 at 64..127 of free column block a. A single
    [128, 81] lhsT then carries both heads.
  - QK_s: rhs is a host-built block-diagonal [128, 84] ([K_even|-sum/81]
    for head 2a in the top-left, head 2a+1 bottom-right) -> one matmul
    yields both heads' [81, 42] score blocks.
  - scores^T: lhsT = stacked K pair, rhs = device-built block-diagonal
    Q_red^T [128, 20] (gathered top-10 query rows, PE-transposed into
    parity halves of a zero-initialized tile) -> [81, 20] for two pairs,
    batched 12 pairs per PSUM bank; bias-added, PE-transposed and
    softmaxed in batch (ACT exp with fused scale/bias/accum-sum).
  - upd^T: lhsT = two heads' V [81, 128], rhs = normalized attn^T [81, 20];
    the off-diagonal [64, 10] quadrants of the [128, 20] output are unused
    garbage, diagonal quadrants are the two pairs' upd^T.
  - ctx cumsum: one lower-triangular matmul per b (L^T(tri) @ V).
  - top-10 per pair via DVE max8/max_index/match_replace (two passes) on a
    PE-transposed [128pairs, 81] M matrix, per 16-b chunk.
  - attn/upd leave the device transposed, one bulk DMA per chunk; the host
    un-transposes during unsharding and scatters upd rows into ctx.
Inputs are shipped partition-major ([128, bs*w]) so multi-batch loads are
a single large-descriptor DMA.
"""

import sys

sys.path.insert(0, "/opt/trn_rl_repo")

from contextlib import ExitStack

import numpy as np

from concourse import bacc, bass, mybir, tile
from concourse.bass import IndirectOffsetOnAxis
from concourse.bass_utils import run_bass_kernel_spmd
from concourse.masks import make_identity

B, L, H, D = 1024, 81, 8, 64
NCORES = 8
WS = 9
U = 10          # n_top queries
S = 41          # sampled (even) keys
SX = S + 1      # + folded -sum/81 column
HD = H * D      # 512
HG = H // 2     # head-pair blocks
GMAX = 12       # pairs per softmax group (12*10=120 <= 128 partitions)

F32 = mybir.dt.float32
I32 = mybir.dt.int32
U32 = mybir.dt.uint32
AX = mybir.AxisListType
ACTF = mybir.ActivationFunctionType


def _rel_pos_index(ws):
    coords = np.stack(np.meshgrid(np.arange(ws), np.arange(ws), indexing="ij"))
    cf = coords.reshape(2, -1)
    rel = (cf[:, :, None] - cf[:, None, :]).transpose(1, 2, 0)
    rel[..., 0] += ws - 1
    rel[..., 1] += ws - 1
    rel[..., 0] *= 2 * ws - 1
    return rel.sum(-1)


def build_program(bs):
    """Build the SPMD Bass program for a per-core shard of `bs` batches."""
    ch_b = min(16, bs)          # batches per chunk
    assert bs % ch_b == 0 and ch_b % 4 == 0
    nch = bs // ch_b
    P = ch_b * H                # (b, h) pairs per chunk (<= 128)
    NB = 4                      # batches per load DMA

    nc = bacc.Bacc("TRN2", target_bir_lowering=False, debug=False,
                   num_devices=NCORES)

    qt_d = nc.dram_tensor("qt_in", [128, bs * HG * L], F32, kind="ExternalInput").ap()
    ktx_d = nc.dram_tensor("ktx_in", [128, bs * HG * 2 * SX], F32,
                           kind="ExternalInput").ap()
    kt_d = nc.dram_tensor("kt_in", [128, bs * HG * L], F32, kind="ExternalInput").ap()
    v_d = nc.dram_tensor("v_in", [L, bs * HD], F32, kind="ExternalInput").ap()
    vt_d = nc.dram_tensor("vt_in", [128, bs * HG * L], F32, kind="ExternalInput").ap()
    qg_d = nc.dram_tensor("qg_in", [bs * H * L, D], F32, kind="ExternalInput").ap()
    rpbt_d = nc.dram_tensor("rpbt_in", [L, U * GMAX], F32, kind="ExternalInput").ap()

    ctx_d = nc.dram_tensor("ctx_out", [128, bs * HG * L], F32,
                           kind="ExternalOutput").ap()
    attn_d = nc.dram_tensor("attn_out", [nch, L, P * U], F32,
                            kind="ExternalOutput").ap()
    upd_d = nc.dram_tensor("upd_out", [nch, D, P * U], F32,
                           kind="ExternalOutput").ap()
    idx_d = nc.dram_tensor("idx_out", [bs * H, U], F32, kind="ExternalOutput").ap()

    W_Q = HG * L                # 324 free cols per batch (qt/kt)
    W_X = HG * 2 * SX           # 336 free cols per batch (ktx, block-diag)

    with tile.TileContext(nc) as tc, ExitStack() as ctx:
        pool = lambda name, bufs, space="SBUF": ctx.enter_context(
            tc.tile_pool(name=name, bufs=bufs, space=space))

        const_p = pool("const", 1)
        qt_p = pool("qt", 3)
        ktx_p = pool("ktx", 3)
        kt_p = pool("kt", ch_b // 4 + 2)
        v_p = pool("v", ch_b // 4 + 2)
        vt_p = pool("vt", 3)
        ctxT_p = pool("ctxT", 3)
        mx_p = pool("mx", 3)
        mc_p = pool("mc", 2)
        tk_p = pool("tk", 2)
        gath_p = pool("gath", 2)
        qrt_p = pool("qrt", 1)
        sm_p = pool("sm", 4)
        och_p = pool("och", 2)

        qks_pp = pool("qks_pp", 2, "PSUM")
        grp_pp = pool("grp_pp", 6, "PSUM")

        ident = const_p.tile([128, 128], F32, tag="ident")
        make_identity(nc, ident[:])
        rpbt = const_p.tile([L, U * GMAX], F32, tag="rpbt")
        nc.sync.dma_start(rpbt[:], rpbt_d[:])
        zer = const_p.tile([128, L], F32, tag="zer")
        nc.gpsimd.memset(zer[:], 0.0)

        # Persistent block-diagonal Q_red^T tiles (zero halves live forever).
        q2t = [qrt_p.tile([128, U * 128], F32, tag="qrt%d" % i,
                          name="qrt%d" % i) for i in range(2)]
        for t in q2t:
            nc.gpsimd.memset(t[:], 0.0)

        for c in range(nch):
            mcols = mc_p.tile([L, 128], F32, tag="mcols")
            kts = []
            vbs = []
            for b4 in range(ch_b // NB):
                b = c * ch_b + NB * b4
                qt = qt_p.tile([128, NB * W_Q], F32, tag="qt")
                nc.sync.dma_start(qt[:], qt_d[:, b * W_Q:(b + NB) * W_Q])
                ktx = ktx_p.tile([128, NB * W_X], F32, tag="ktx")
                nc.sync.dma_start(ktx[:], ktx_d[:, b * W_X:(b + NB) * W_X])
                kt = kt_p.tile([128, NB * W_Q], F32, tag="kt")
                nc.sync.dma_start(kt[:], kt_d[:, b * W_Q:(b + NB) * W_Q])
                vb = v_p.tile([L, NB * HD], F32, tag="v")
                nc.sync.dma_start(vb[:], v_d[:, b * HD:(b + NB) * HD])
                vt = vt_p.tile([128, NB * W_Q], F32, tag="vt")
                nc.sync.dma_start(vt[:], vt_d[:, b * W_Q:(b + NB) * W_Q])
                ctxT = ctxT_p.tile([128, NB * W_Q], F32, tag="ctxT")
                kts.append(kt)
                vbs.append(vb)

                for bi in range(NB):
                    bl = NB * b4 + bi
                    # Phase 1: one matmul per head pair -> [81, 84]
                    qks = qks_pp.tile([L, H * SX], F32, tag="qks")
                    for a in range(HG):
                        nc.tensor.matmul(
                            qks[:, a * 2 * SX:(a + 1) * 2 * SX],
                            lhsT=qt[:, bi * W_Q + a * L:bi * W_Q + (a + 1) * L],
                            rhs=ktx[:, bi * W_X + a * 2 * SX:
                                    bi * W_X + (a + 1) * 2 * SX],
                            start=True, stop=True)
                    qksv = qks[:].rearrange("p (h s) -> p h s", s=SX)
                    mxt = mx_p.tile([L, H], F32, tag="mxt")
                    nc.vector.reduce_max(mxt[:], qksv[:, :, 0:S], axis=AX.X)
                    nc.vector.tensor_add(
                        mcols[:, bl * H:(bl + 1) * H], mxt[:], qksv[:, :, S])

                    # causal cumsum of V: DVE prefix scan per head-pair col
                    for a in range(HG):
                        off = bi * W_Q + a * L
                        nc.vector.tensor_tensor_scan(
                            ctxT[:, off:off + L], vt[:, off:off + L], zer[:],
                            initial=0.0, op0=mybir.AluOpType.add,
                            op1=mybir.AluOpType.add)
                nc.sync.dma_start(ctx_d[:, b * W_Q:(b + NB) * W_Q], ctxT[:])

            # ---- top-10 per pair over the chunk ----
            mt_ps = grp_pp.tile([128, L], F32, tag="grp")
            nc.tensor.transpose(mt_ps[:P, :], mcols[:, :P], ident[:L, :L])
            xsb = tk_p.tile([128, L], F32, tag="xsb")
            nc.vector.tensor_copy(xsb[:P], mt_ps[:P])
            mx8 = tk_p.tile([128, 8], F32, tag="mx8")
            nc.vector.max(out=mx8[:P], in_=xsb[:P])
            idx1 = tk_p.tile([128, 8], U32, tag="idx1")
            nc.vector.max_index(idx1[:P], mx8[:P], xsb[:P])
            x2 = tk_p.tile([128, L], F32, tag="x2")
            nc.vector.match_replace(out=x2[:P], in_to_replace=mx8[:P],
                                    in_values=xsb[:P], imm_value=-1e30)
            mx8b = tk_p.tile([128, 8], F32, tag="mx8b")
            nc.vector.max(out=mx8b[:P], in_=x2[:P])
            idx2 = tk_p.tile([128, 8], U32, tag="idx2")
            nc.vector.max_index(idx2[:P], mx8b[:P], x2[:P])
            idxf = tk_p.tile([128, U], F32, tag="idxf")
            nc.vector.tensor_copy(idxf[:P, 0:8], idx1[:P])
            nc.vector.tensor_copy(idxf[:P, 8:U], idx2[:P, 0:2])
            nc.sync.dma_start(idx_d[c * P:(c + 1) * P], idxf[:P])

            # gather offsets: row = (b*H + h)*L + idx  (pair-major shard rows)
            rowb_i = tk_p.tile([128, 1], I32, tag="rowbi")
            nc.gpsimd.iota(rowb_i[:P], pattern=[[0, 1]], base=c * P * L,
                           channel_multiplier=L)
            rowb_f = tk_p.tile([128, 1], F32, tag="rowbf")
            nc.vector.tensor_copy(rowb_f[:P], rowb_i[:P])
            offf = tk_p.tile([128, U], F32, tag="offf")
            nc.vector.tensor_scalar_add(offf[:P], idxf[:P], rowb_f[:P, 0:1])
            offi = tk_p.tile([128, U], I32, tag="offi")
            nc.vector.tensor_copy(offi[:P], offf[:P])

            gath = gath_p.tile([128, U * D], F32, tag="gath")
            for j in range(U):
                nc.gpsimd.indirect_dma_start(
                    out=gath[:P, j * D:(j + 1) * D],
                    out_offset=None,
                    in_=qg_d[:],
                    in_offset=IndirectOffsetOnAxis(ap=offi[:P, j:j + 1], axis=0),
                )
            # Q_red^T -> block-diagonal tile: pair p's 10 cols at p*10, rows
            # (p%2)*64..+64; the other half stays zero.
            qredT = q2t[c % 2]
            qrv = qredT[:].rearrange("p (a c) -> p a c", c=2 * U)
            for j in range(U):
                tp = grp_pp.tile([D, 128], F32, tag="grp")
                nc.tensor.transpose(tp[:, :P], gath[:P, j * D:(j + 1) * D],
                                    ident[:P, :P])
                tpv = tp[:].rearrange("d (a t) -> d a t", t=2)
                for par in range(2):
                    nc.scalar.copy(
                        qrv[par * D:(par + 1) * D, :P // 2, par * U + j],
                        tpv[:, :P // 2, par])

            # chunk-level transposed outputs (one DMA each per chunk)
            atT_ch = och_p.tile([L, P * U], F32, tag="atT")
            updT_ch = och_p.tile([D, P * U], F32, tag="updT")

            # ---- softmax / upd over groups of pairs ----
            p0 = 0
            while p0 < P:
                gn = min(GMAX, P - p0)
                rows = gn * U

                sct = grp_pp.tile([L, U * GMAX], F32, tag="grp")
                for a in range(gn // 2):
                    p = p0 + 2 * a
                    bl, h = divmod(p, H)
                    m = h // 2
                    nc.tensor.matmul(
                        sct[:, a * 2 * U:(a + 1) * 2 * U],
                        lhsT=kts[bl // NB][:, (bl % NB) * W_Q + m * L:
                                           (bl % NB) * W_Q + (m + 1) * L],
                        rhs=qredT[:, p * U:(p + 2) * U],
                        start=True, stop=True)
                tmpT = sm_p.tile([L, U * GMAX], F32, tag="tmpT")
                nc.vector.tensor_add(tmpT[:, :rows], sct[:, :rows],
                                     rpbt[:, :rows])
                str_ps = grp_pp.tile([U * GMAX, L], F32, tag="grp")
                nc.tensor.transpose(str_ps[:rows, :], tmpT[:, :rows],
                                    ident[:L, :L])
                mxg = sm_p.tile([U * GMAX, 1], F32, tag="mxg")
                nc.vector.reduce_max(mxg[:rows], str_ps[:rows], axis=AX.X)
                nmx = sm_p.tile([U * GMAX, 1], F32, tag="nmx")
                nc.vector.tensor_scalar_mul(nmx[:rows], mxg[:rows], -0.125)
                attne = sm_p.tile([U * GMAX, L], F32, tag="attne")
                ssum = sm_p.tile([U * GMAX, 1], F32, tag="ssum")
                nc.scalar.activation(attne[:rows], str_ps[:rows], ACTF.Exp,
                                     bias=nmx[:rows, 0:1], scale=0.125,
                                     accum_out=ssum[:rows, 0:1])
                rinv = sm_p.tile([U * GMAX, 1], F32, tag="rinv")
                nc.vector.reciprocal(rinv[:rows], ssum[:rows])
                attno = sm_p.tile([U * GMAX, L], F32, tag="attno")
                nc.vector.tensor_scalar_mul(attno[:rows], attne[:rows],
                                            rinv[:rows, 0:1])

                atT_ps = grp_pp.tile([L, U * GMAX], F32, tag="grp")
                nc.tensor.transpose(atT_ps[:, :rows], attno[:rows, :],
                                    ident[:rows, :rows])
                nc.scalar.copy(atT_ch[:, p0 * U:p0 * U + rows],
                               atT_ps[:, :rows])

                updT_ps = grp_pp.tile([128, U * GMAX], F32, tag="grp")
                for a in range(gn // 2):
                    p = p0 + 2 * a
                    bl, h = divmod(p, H)
                    nc.tensor.matmul(
                        updT_ps[:, a * 2 * U:(a + 1) * 2 * U],
                        lhsT=vbs[bl // NB][:, (bl % NB) * HD + h * D:
                                           (bl % NB) * HD + (h + 2) * D],
                        rhs=atT_ch[:, p * U:(p + 2) * U],
                        start=True, stop=True)
                # diagonal quadrants -> updT_ch; off-diagonals are garbage
                upv = updT_ps[:, :rows].rearrange("d (a t u) -> d t a u",
                                                  t=2, u=U)
                ucv = updT_ch[:, p0 * U:p0 * U + rows].rearrange(
                    "d (a t u) -> d t a u", t=2, u=U)
                for par in range(2):
                    nc.scalar.copy(ucv[:, par],
                                   upv[par * D:(par + 1) * D, par])
                p0 += gn

            nc.sync.dma_start(attn_d[c], atT_ch[:])
            nc.sync.dma_start(upd_d[c], updT_ch[:])

    nc.compile()
    return nc


_PROG_CACHE = {}


def _get_prog(bs):
    if bs not in _PROG_CACHE:
        _PROG_CACHE[bs] = build_program(bs)
    return _PROG_CACHE[bs]


def _pair_stack(a):
    """[B, d, h, w] -> [B, 2d, (h//2)*w]: head 2a+par at rows par*d, col a*w."""
    b, d, h, w = a.shape
    return np.ascontiguousarray(
        a.transpose(0, 2, 1, 3).reshape(b, h // 2, 2, d, w)
        .transpose(0, 2, 3, 1, 4)).reshape(b, 2 * d, (h // 2) * w)


def make_in_maps(q, k, v, bt, ncores):
    """Host-side layout prep + sharding. Returns list of per-core input dicts."""
    b_tot = q.shape[0]
    bs = b_tot // ncores
    qtf = np.ascontiguousarray(q.transpose(0, 3, 2, 1))        # [B, D, H, L]
    ktf = np.ascontiguousarray(k.transpose(0, 3, 2, 1))        # [B, D, H, L]
    keven = ktf[:, :, :, 0::2]                                  # [B, D, H, 41]
    ksum = -keven.sum(-1, keepdims=True) / np.float32(L)
    ktxf = np.concatenate([keven, ksum], -1)                    # [B, D, H, 42]
    qt = _pair_stack(qtf)                                       # [B, 128, 324]
    kt = _pair_stack(ktf)
    # block-diagonal [B, 128, HG*2*SX]: head 2a top-left, 2a+1 bottom-right
    ktx = np.zeros((b_tot, 2, D, HG, 2, SX), np.float32)
    ktx[:, 0, :, :, 0, :] = ktxf[:, :, 0::2].transpose(0, 1, 2, 3)
    ktx[:, 1, :, :, 1, :] = ktxf[:, :, 1::2]
    ktx = ktx.reshape(b_tot, 128, HG * 2 * SX)
    vr = v.reshape(b_tot, L, HD)
    vt = _pair_stack(np.ascontiguousarray(v.transpose(0, 3, 2, 1)))
    qg = np.ascontiguousarray(q.transpose(0, 2, 1, 3)).reshape(b_tot * H * L, D)

    rel = _rel_pos_index(WS)
    rpb = bt[rel.ravel(), 0].reshape(L, L)[:U, :]               # [10, 81]
    rpbt = np.ascontiguousarray(np.tile(rpb.T, (1, GMAX)))      # [81, 120]

    in_maps = []
    for c in range(ncores):
        sl = slice(c * bs, (c + 1) * bs)
        in_maps.append({
            "qt_in": np.ascontiguousarray(
                qt[sl].transpose(1, 0, 2)).reshape(128, bs * HG * L),
            "ktx_in": np.ascontiguousarray(
                ktx[sl].transpose(1, 0, 2)).reshape(128, bs * HG * 2 * SX),
            "kt_in": np.ascontiguousarray(
                kt[sl].transpose(1, 0, 2)).reshape(128, bs * HG * L),
            "v_in": np.ascontiguousarray(
                vr[sl].transpose(1, 0, 2)).reshape(L, bs * HD),
            "vt_in": np.ascontiguousarray(
                vt[sl].transpose(1, 0, 2)).reshape(128, bs * HG * L),
            "qg_in": qg[c * bs * H * L:(c + 1) * bs * H * L],
            "rpbt_in": rpbt,
        })
    return in_maps, bs


def assemble(results, ncores, bs):
    """Host-side unsharding, un-transposing, and top-10 scatter-merge."""
    b_tot = ncores * bs
    ch_b = min(16, bs)
    nch = bs // ch_b
    P = ch_b * H
    ctx_full = np.empty((b_tot, L, H, D), np.float32)
    attn_full = np.empty((b_tot, H, U, L), np.float32)
    bi = np.arange(bs)[:, None, None]
    hi = np.arange(H)[None, :, None]
    for c in range(ncores):
        r = results[c]
        # ctx_out [128, bs*HG*L]: partition (par, d), col (b, a, l)
        cs = np.ascontiguousarray(
            np.asarray(r["ctx_out"]).reshape(2, D, bs, HG, L)
            .transpose(2, 4, 3, 0, 1)).reshape(bs, L, H, D)
        # [nch, L, P, U] -> [nch, P, U, L] -> [bs, H, U, L]
        at = np.asarray(r["attn_out"]).reshape(nch, L, P, U) \
            .transpose(0, 2, 3, 1).reshape(bs, H, U, L)
        ud = np.asarray(r["upd_out"]).reshape(nch, D, P, U) \
            .transpose(0, 2, 3, 1).reshape(bs, H, U, D)
        ix = np.rint(r["idx_out"]).astype(np.int64).reshape(bs, H, U)
        cs[bi, ix, hi] = ud
        ctx_full[c * bs:(c + 1) * bs] = cs
        attn_full[c * bs:(c + 1) * bs] = at
    return ctx_full, attn_full


def kernel(queries, keys, values, bias_table, attn_mask=None, _trace=False):
    q = np.ascontiguousarray(np.asarray(queries, dtype=np.float32))
    k = np.ascontiguousarray(np.asarray(keys, dtype=np.float32))
    v = np.ascontiguousarray(np.asarray(values, dtype=np.float32))
    bt = np.asarray(bias_table, dtype=np.float32)

    in_maps, bs = make_in_maps(q, k, v, bt, NCORES)
    nc = _get_prog(bs)
    res = run_bass_kernel_spmd(nc, in_maps, list(range(NCORES)), trace=_trace)
    out = assemble(res.results, NCORES, bs)
    if _trace:
        return out, res
    return out
